# revision 21
# baseline (speedup 1.0000x reference)
"""AGCRN cell with per-node MLP-generated gate weights, on 8 TRN2 NeuronCores.

Math (reference):
    combined = adj @ concat([x, h], -1)          # [N, 257]
    cg = combined[nodes_ind]                     # [M, 257]
    gate(f, q, W, b) = einsum('ni,nd,dio->no', f, q, W) + q @ b
    r = sigmoid(gate(cg, q, W_r, b_r)); u = sigmoid(gate(cg, q, W_u, b_u))
    cn = [x_sel, r * h_sel]                      # [M, 257]
    cand = tanh(gate(cn, q, W_c, b_c))
    new_h = (1 - u) * (r * h_sel) + u * cand     # [M, 128]

Every output row depends only on its own (adj-row, q-row, x-row, h-row), so the
M rows shard cleanly across the 8 cores with W_*/C replicated — no collectives.

Per-core device graph (Mc = 256 rows, transposed orientation [feat, n]):
    cg^T [256, 256] = C^T @ A_sel^T       (PE, K=4096 tiled by 128, 2 m-groups)
    z^T[(d,i), n]   = q[n,d] * f^T[i, n]  (DVE/GpSimd broadcast-AP multiply
                                           against a partition-broadcast q_bcast)
    gate^T [128, 256] = W_flat^T @ z^T + b^T @ q^T   (PE, K tiled by 128)
    sigmoid/tanh on ACT, final elementwise combine on DVE, f32 out.

Feature bookkeeping: all three gates use the same device feature order
[x cols 0..127, (h or r*h) cols 0..127] (256 features = 2 clean m-groups) plus
one leftover feature (x col 128).  For r/u its mixed value adj@x[:,128] is a
cheap host-side matvec (input "cgL"); for c it is x_sel[:,128] ("xtail").
W_flat is permuted to match ([0..127, 129..256] main + row 128 tail).

All large DRAM tensors are pre-swizzled host-side into partition-major layout
([128, k_tiles*width], contiguous per partition) so every big DMA moves large
bursts.  Loads are chunked into separate tiles so matmuls start as soon as
their chunk lands, and are balanced across the sync/scalar HWDGE queues with
the critical C/A^T chunks first.  q_bcast is built by a GpSimd
partition-broadcast from a 16KB row, costing no DMA/PE/ACT time.
"""

import os
import sys

sys.path.insert(0, "/opt/trn_rl_repo")

import numpy as np
from ml_dtypes import bfloat16

import concourse.bass as bass
import concourse.tile as tile
from concourse import bacc, mybir
from concourse.bass_utils import run_bass_kernel_spmd

NC = 8
N = 4096
M = 2048
Mc = M // NC  # 256 rows per core
F = 257  # feature dim of concat([x, h])
FD = 256  # device feature dim (leftover x col 128 handled via cgL/xtail)
QD = 32
O = 128  # output dim
KT = 64  # full 128-row K-tiles of the (d, i<256) contraction
KROWS = QD * 256  # 8192
CHUNKS = [4, 10, 14, 4]  # C/AT chunks: small first (early PE start), small last (short tail)
ZCH = 4  # chunks for z construction (8 d's each)
SMW = 3 * 256 + 6 * 128  # packed [32, x] smalls: qT, b_r/u/c, xtail, cgL, Wt_r/u/c
BF16 = mybir.dt.bfloat16
F32 = mybir.dt.float32
AF = mybir.ActivationFunctionType

_COMPILED = None  # compile once per process


def _build():
    nc = bacc.Bacc("TRN2", target_bir_lowering=False, debug=False, num_devices=NC)
    d_AT = nc.dram_tensor("AT", [128, 32 * Mc], BF16, kind="ExternalInput").ap()
    d_C = nc.dram_tensor("C", [128, 32 * FD], BF16, kind="ExternalInput").ap()
    d_SM = nc.dram_tensor("SM", [QD, SMW], BF16, kind="ExternalInput").ap()
    d_qTf = nc.dram_tensor("qTflat", [1, KROWS], BF16, kind="ExternalInput").ap()
    d_xT = nc.dram_tensor("xT", [128, Mc], BF16, kind="ExternalInput").ap()
    d_hT = nc.dram_tensor("hT", [128, Mc], F32, kind="ExternalInput").ap()
    d_W = {
        g: nc.dram_tensor(f"W{g}", [128, KT * O], BF16, kind="ExternalInput").ap()
        for g in "ruc"
    }
    d_out = nc.dram_tensor("out", [O, Mc], F32, kind="ExternalOutput").ap()

    WW = KT // 2 * O  # W half width (32 k-tiles = 16 d's)
    ZW = KROWS // ZCH  # z chunk width (8 d's)

    with tile.TileContext(nc) as tc:
        with (
            tc.tile_pool(name="res", bufs=1) as res,
            tc.tile_pool(name="psum", bufs=1, space=bass.MemorySpace.PSUM) as pp,
        ):
            # --- ACT table preload (sigmoid_and_others holds copy/sigmoid/tanh)
            warm = res.tile([1, 8], F32, name="warm")
            nc.vector.memset(warm[:], 0.0)
            warm2 = res.tile([1, 8], F32, name="warm2")
            nc.scalar.activation(warm2[:], warm[:], AF.Sigmoid)
            # GpSimd ucode preload: tiny dummy partition_broadcast
            warm3 = res.tile([1, 64], BF16, name="warm3")
            nc.vector.memset(warm3[:], 0.0)
            warm4 = res.tile([128, 64], BF16, name="warm4")
            nc.gpsimd.partition_broadcast(warm4[:], warm3[:])

            # --- sync queue first: qTflat feeds the on-chip q_bcast halves
            qTf_sb = res.tile([1, KROWS], BF16, name="qTf_sb")
            nc.sync.dma_start(qTf_sb[:], d_qTf[:])
            q_bc2 = [
                res.tile([128, KROWS // 2], BF16, name=f"q_bc{hh}") for hh in range(2)
            ]
            for hh in range(2):
                nc.gpsimd.partition_broadcast(
                    q_bc2[hh][:], qTf_sb[:, hh * (KROWS // 2) : (hh + 1) * (KROWS // 2)]
                )
            # ramped chunk sizes: small first so PE gets work early (HAM warm)
            C_sb, AT_sb = [], []
            off = 0
            for j, w in enumerate(CHUNKS):
                ct = res.tile([128, w * FD], BF16, name=f"C_sb{j}")
                nc.sync.dma_start(ct[:], d_C[:, off * FD : (off + w) * FD])
                C_sb.append(ct)
                at = res.tile([128, w * Mc], BF16, name=f"AT_sb{j}")
                nc.scalar.dma_start(at[:], d_AT[:, off * Mc : (off + w) * Mc])
                AT_sb.append(at)
                off += w

            # packed smalls + xT/hT land after C (needed from the gate phase)
            sm_sb = res.tile([QD, SMW], BF16, name="sm_sb")
            nc.sync.dma_start(sm_sb[:], d_SM[:])
            xT_sb = res.tile([128, Mc], BF16, name="xT_sb")
            nc.sync.dma_start(xT_sb[:], d_xT[:])
            hT_sb = res.tile([128, Mc], F32, name="hT_sb")
            nc.sync.dma_start(hT_sb[:], d_hT[:])
            qT_sb = sm_sb[:, 0:Mc]
            b_sb = {g: sm_sb[:, Mc + i * O : Mc + (i + 1) * O] for i, g in enumerate("ruc")}
            xtail_sb = sm_sb[:, Mc + 3 * O : 2 * Mc + 3 * O]
            cgL_sb = sm_sb[:, 2 * Mc + 3 * O : 3 * Mc + 3 * O]
            Wt_sb = {
                g: sm_sb[:, 3 * Mc + (3 + i) * O : 3 * Mc + (4 + i) * O]
                for i, g in enumerate("ruc")
            }
            W_sb = {g: [None, None] for g in "ruc"}
            for g, half, eng in (
                ("r", 0, nc.sync),
                ("u", 0, nc.scalar),
                ("r", 1, nc.sync),
                ("u", 1, nc.scalar),
                ("c", 0, nc.scalar),
                ("c", 1, nc.sync),
            ):
                wt = res.tile([128, WW], BF16, name=f"W{g}_sb{half}")
                eng.dma_start(wt[:], d_W[g][:, half * WW : (half + 1) * WW])
                W_sb[g][half] = wt


            # --- gate bias matmuls first (opens the psum groups), then HAM
            # warm-up fillers so the adj matmuls run at full clock
            pr = pp.tile([128, Mc], F32, name="pr")
            pu = pp.tile([128, Mc], F32, name="pu")
            pc = pp.tile([128, Mc], F32, name="pc")
            nc.tensor.matmul(pr[:], b_sb["r"], qT_sb, start=True, stop=False)
            nc.tensor.matmul(pu[:], b_sb["u"], qT_sb, start=True, stop=False)
            nc.tensor.matmul(pc[:], b_sb["c"], qT_sb, start=True, stop=False)

            # --- phase 1: cg^T = C^T @ A_sel^T (2 m-groups, chunk-major)
            pcg = [
                pp.tile([128, Mc], F32, name="pcg0"),
                pp.tile([128, Mc], F32, name="pcg1"),
            ]
            base = 0
            for j, w in enumerate(CHUNKS):
                for gi in range(2):
                    for tt in range(w):
                        t = base + tt
                        nc.tensor.matmul(
                            pcg[gi][:, :],
                            C_sb[j][:, tt * FD + gi * 128 : tt * FD + gi * 128 + 128],
                            AT_sb[j][:, tt * Mc : (tt + 1) * Mc],
                            start=(t == 0),
                            stop=(t == 31),
                        )
                base += w
            cgT = [
                res.tile([128, Mc], BF16, name="cgT0"),
                res.tile([128, Mc], BF16, name="cgT1"),
            ]
            for i in range(2):
                nc.scalar.activation(cgT[i][:], pcg[i][:], AF.Copy)

            # --- phase 2: z chunks. zg on DVE; input-only zc parts on GpSimd
            def zchunks(dst_list, src_ap, engine, ks=range(ZCH)):
                # dst[k][p, dd*Mc + n] = src[p, n] * q[n, k*8 + dd]
                ZH = KROWS // 2
                for k in ks:
                    off = k * ZW
                    qsl = q_bc2[off // ZH][:, off % ZH : off % ZH + ZW]
                    engine.tensor_mul(
                        dst_list[k][:].rearrange("p (a b) -> p a b", b=Mc),
                        src_ap.unsqueeze(1).broadcast_to((128, QD // ZCH, Mc)),
                        qsl.rearrange("p (a b) -> p a b", b=Mc),
                    )

            zgt = res.tile([QD, Mc], BF16, name="zgt")
            nc.vector.tensor_mul(zgt[:], qT_sb, cgL_sb)
            zct = res.tile([QD, Mc], BF16, name="zct")
            nc.vector.tensor_mul(zct[:], qT_sb, xtail_sb)
            zc0 = [res.tile([128, ZW], BF16, name=f"zc0_{k}") for k in range(ZCH)]
            zg0 = [res.tile([128, ZW], BF16, name=f"zg0_{k}") for k in range(ZCH)]
            zg1 = [res.tile([128, ZW], BF16, name=f"zg1_{k}") for k in range(ZCH)]
            for k in range(ZCH):
                zchunks(zg0, cgT[0][:], nc.vector, ks=[k])
                zchunks(zg1, cgT[1][:], nc.vector, ks=[k])
                if k >= 2:
                    zchunks(zc0, xT_sb[:], nc.vector, ks=[k - 2])
            zchunks(zc0, xT_sb[:], nc.vector, ks=[2, 3])

            # --- phase 3: gates r, u (k-tile t -> d = t//2, ihalf = t%2)
            def zslice(zl, d):
                k, dd = d // (QD // ZCH), d % (QD // ZCH)
                return zl[k][:, dd * Mc : (dd + 1) * Mc]

            for t in range(KT):
                d, ih = t // 2, t % 2
                zsl = zslice(zg0 if ih == 0 else zg1, d)
                wsl = slice((t % 32) * O, (t % 32 + 1) * O)
                nc.tensor.matmul(pr[:], W_sb["r"][t // 32][:, wsl], zsl, start=False, stop=False)
                nc.tensor.matmul(pu[:], W_sb["u"][t // 32][:, wsl], zsl, start=False, stop=False)
            nc.tensor.matmul(pr[:], Wt_sb["r"], zgt[:], start=False, stop=True)
            nc.tensor.matmul(pu[:], Wt_sb["u"], zgt[:], start=False, stop=True)

            r_sb = res.tile([128, Mc], F32, name="r_sb")
            nc.scalar.activation(r_sb[:], pr[:], AF.Sigmoid)
            u_sb = res.tile([128, Mc], F32, name="u_sb")
            nc.scalar.activation(u_sb[:], pu[:], AF.Sigmoid)

            # --- gate c, x-part (fills PE while sigmoid/rh/zc1 run)
            for t in range(0, KT, 2):  # ihalf = 0 tiles
                d = t // 2
                nc.tensor.matmul(
                    pc[:],
                    W_sb["c"][t // 32][:, (t % 32) * O : (t % 32 + 1) * O],
                    zslice(zc0, d),
                    start=False,
                    stop=False,
                )
            nc.tensor.matmul(pc[:], Wt_sb["c"], zct[:], start=False, stop=False)

            # --- phase 4: rh, then zc1 chunks interleaved with gate-c h-part
            rh_f = res.tile([128, Mc], F32, name="rh_f")
            nc.vector.tensor_mul(rh_f[:], r_sb[:], hT_sb[:])
            rh_b = res.tile([128, Mc], BF16, name="rh_b")
            nc.vector.tensor_copy(rh_b[:], rh_f[:])
            zc1 = [res.tile([128, ZW], BF16, name=f"zc1_{k}") for k in range(ZCH)]
            for k in range(ZCH):
                zchunks(zc1, rh_b[:], nc.vector, ks=[k])
                for dd in range(QD // ZCH):
                    d = k * (QD // ZCH) + dd
                    t = 2 * d + 1
                    nc.tensor.matmul(
                        pc[:],
                        W_sb["c"][t // 32][:, (t % 32) * O : (t % 32 + 1) * O],
                        zslice(zc1, d),
                        start=False,
                        stop=(t == KT - 1),
                    )
            cand_sb = res.tile([128, Mc], F32, name="cand_sb")
            nc.scalar.activation(cand_sb[:], pc[:], AF.Tanh)

            # --- phase 6: new_h^T = rh + u * (cand - rh)
            t1 = res.tile([128, Mc], F32, name="t1")
            nc.vector.tensor_sub(t1[:], cand_sb[:], rh_f[:])
            t2 = res.tile([128, Mc], F32, name="t2")
            nc.vector.tensor_mul(t2[:], u_sb[:], t1[:])
            outT = res.tile([128, Mc], F32, name="outT")
            nc.vector.tensor_add(outT[:], rh_f[:], t2[:])
            nc.sync.dma_start(d_out[:], outT[:])

    nc.compile()
    return nc


def _get_compiled():
    global _COMPILED
    if _COMPILED is None:
        _COMPILED = _build()
    return _COMPILED


def _pmajor(a, width):
    """[n_tiles*128, width] row-major -> [128, n_tiles*width] partition-major."""
    nt = a.shape[0] // 128
    return np.ascontiguousarray(
        a.reshape(nt, 128, width).transpose(1, 0, 2).reshape(128, nt * width)
    )


def _prep_inputs(x, h, query_vectors, adj, nodes_ind, W_u, b_u, W_r, b_r, W_c, b_c):
    idx = np.asarray(nodes_ind).astype(np.int64)
    f32 = np.float32
    x = np.asarray(x, f32)
    h = np.asarray(h, f32)
    q = np.asarray(query_vectors, f32)
    adj = np.asarray(adj, f32)

    C_dev = np.concatenate([x[:, :128], h], axis=1).astype(bfloat16)  # [N, 256]
    C_pm = _pmajor(C_dev, FD)  # [128, 32*256]
    A_sel = adj[idx]  # [M, N]
    x_sel = x[idx]
    h_sel = h[idx]
    cg_L = A_sel @ x[:, 128]  # [M] — the leftover mixed feature, on host

    def flatW(W):
        W = np.asarray(W, f32)
        main = (
            np.concatenate([W[:, :128, :], W[:, 129:, :]], axis=1)
            .reshape(QD * 256, O)
            .astype(bfloat16)
        )
        tail = np.ascontiguousarray(W[:, 128, :]).astype(bfloat16)
        return _pmajor(main, O), tail  # [128, 64*128], [32, 128]

    Wf = {"r": flatW(W_r), "u": flatW(W_u), "c": flatW(W_c)}
    bf = {
        "r": np.asarray(b_r, f32).astype(bfloat16),
        "u": np.asarray(b_u, f32).astype(bfloat16),
        "c": np.asarray(b_c, f32).astype(bfloat16),
    }

    in_maps = []
    for c in range(NC):
        sl = slice(c * Mc, (c + 1) * Mc)
        qT = np.ascontiguousarray(q[sl].T).astype(bfloat16)  # [32, 256]
        AT = np.ascontiguousarray(A_sel[sl].T).astype(bfloat16)  # [4096, 256]
        # packed [32, x] smalls: qT | b_r | b_u | b_c | xtail | cgL | Wt_r/u/c
        SM = np.concatenate(
            [
                qT,
                bf["r"],
                bf["u"],
                bf["c"],
                np.broadcast_to(x_sel[sl, 128], (QD, Mc)).astype(bfloat16),
                np.broadcast_to(cg_L[sl], (QD, Mc)).astype(bfloat16),
                Wf["r"][1],
                Wf["u"][1],
                Wf["c"][1],
            ],
            axis=1,
        )
        in_maps.append(
            {
                "AT": _pmajor(AT, Mc),
                "C": C_pm,
                "SM": np.ascontiguousarray(SM),
                "qTflat": qT.reshape(1, KROWS).copy(),
                "xT": np.ascontiguousarray(x_sel[sl, :128].T).astype(bfloat16),
                "hT": np.ascontiguousarray(h_sel[sl].T).astype(f32),
                "Wr": Wf["r"][0],
                "Wu": Wf["u"][0],
                "Wc": Wf["c"][0],
            }
        )
    return in_maps


def run(inputs: dict, trace: bool = False):
    nc = _get_compiled()
    in_maps = _prep_inputs(**inputs)
    res = run_bass_kernel_spmd(nc, in_maps, core_ids=list(range(NC)), trace=trace)
    shards = [res.results[c]["out"].T for c in range(NC)]  # each [256, 128]
    out = np.concatenate(shards, axis=0).astype(np.float32)  # [M, 128]
    return out, res


def kernel(**inputs) -> np.ndarray:
    out, _ = run(inputs, trace=bool(os.environ.get("BASS_KERNEL_TRACE")))
    return out
